# revision 34
# baseline (speedup 1.0000x reference)
"""Trainium2 Bass kernel for nn_Attention_57423712748130.

Computation (per batch b):
  X4 = x[b] viewed (C=256, N=4096)   [raw reshape]
  Q4 = silu(BN(q_w @ X4))            (256, 4096)
  KV4 = silu(BN(kv_w @ Y4))          (128, 4096)
  q[n,h,d]  = Q4[n1, n0*256+h*64+d]      n = n1*16+n0
  k[m,h,d]  = KV4[m1, m0*512 + h*64+d]   m = m1*8+m0
  v[m,h,d]  = KV4[m1, m0*512+256+h*64+d]
  att = softmax(q k^T / 8); o = att v
  out rows [h*1024,(h+1)*1024) = O_h @ proj_w.T + proj_b
    where O_h[n2, n3*64+d] = o[4*n2+n3, d]

Sharding: 8 cores = (batch b in 0..3) x (head-pair hp in 0..1); each core
computes heads {2hp, 2hp+1} of batch b = rows [hp*2048,(hp+1)*2048) of out[b].

On-core strategy:
 - conv outputs are computed directly in transposed layout (x/y tiles as the
   matmul stationary), so q^T/k^T slices ([d on partitions]) need no transposes
 - conv bias is applied on VectorE (a K=1 bias matmul costs ~500ns of PE each);
   silu is computed as z*(1+tanh(z/2)) = 2*silu(z) — tanh shares the ACT table
   set with exp (plain Silu thrashes ACT_TABLE_LOADs against Exp); the 2x is
   folded: exp scale 1/32 and a 2.0 fill for the V/ones column cancel it
 - scoresT[m,n] = k^T.T @ q^T in PSUM, one [128,1024] exp per 2-bank psum
 - att@v contracts over m with an extra ones-column on V producing the softmax
   denominators as row 64; reciprocal runs as one [16,64] DVE op per quarter
   (a per-n0 [1,256] reciprocal is 1.75us of iterative divide each)
 - scores/att@v operands are bf16 (PSUM accumulation fp32); convs/proj f32r
   (f32r matmuls must self-load weights so their LDWEIGHTS serializes; bf16
   LDWEIGHTS overlaps ~99% with matmuls). measured end-to-end absmax-rel
   ~3.1e-3, resid_var ~1.5e-6; measured ~155-175us across 8 cores
 - normalization + projection run per q0-quarter, pipelined behind the
   attention of later quarters (grids broadcast via a DRAM bounce); O_h
   columns are kept q0-major so every PSUM evacuation is contiguous; the
   final output DMA un-permutes rows
"""

import ml_dtypes
import numpy as np

B = 4
N_TOK = 4096
C = 256
BN_EPS = 1e-5
ATTN_BF16 = True  # scores/att@v operands in bf16 (PSUM accum stays fp32)

_CACHE = {}


def _build():
    import concourse.bacc as bacc
    import concourse.bass as bass
    import concourse.tile as tile
    from concourse import mybir

    f32 = mybir.dt.float32
    f32r = mybir.dt.float32r
    bf16 = mybir.dt.bfloat16
    adt = bf16 if ATTN_BF16 else f32r
    AF = mybir.ActivationFunctionType

    nc = bacc.Bacc("TRN2", target_bir_lowering=False, debug=False, num_devices=8)

    xq = nc.dram_tensor("xq", [256, 2048], f32, kind="ExternalInput")
    yk = nc.dram_tensor("yk", [256, 1024], f32, kind="ExternalInput")
    yv = nc.dram_tensor("yv", [256, 1024], f32, kind="ExternalInput")
    wq = nc.dram_tensor("wq", [256, 256], f32, kind="ExternalInput")
    bq = nc.dram_tensor("bq", [1, 512], f32, kind="ExternalInput")
    wkv = nc.dram_tensor("wkv", [256, 128], f32, kind="ExternalInput")
    bkv = nc.dram_tensor("bkv", [1, 512], f32, kind="ExternalInput")
    bkvc = nc.dram_tensor("bkvc", [128, 1], f32, kind="ExternalInput")
    wp = nc.dram_tensor("wp", [256, 256], f32, kind="ExternalInput")
    bp = nc.dram_tensor("bp", [1, 512], f32, kind="ExternalInput")
    twod = nc.dram_tensor("twod", [1, 1040], adt, kind="ExternalInput")
    out = nc.dram_tensor("out", [2048, 256], f32, kind="ExternalOutput")
    rscr = nc.dram_tensor("rscr", [2, 4096], f32)
    rraw = nc.dram_tensor("rraw", [2, 4096], f32)

    with tile.TileContext(nc) as tc:
        with (
            tc.tile_pool(name="const", bufs=1) as cp,
            tc.tile_pool(name="actt", bufs=3) as actt,
            tc.tile_pool(name="attp", bufs=16) as attp,
            tc.tile_pool(name="outp", bufs=4) as outp,
            tc.tile_pool(name="nrm", bufs=4) as nrm,
            tc.tile_pool(name="psc", bufs=2, space="PSUM") as psc,
            tc.tile_pool(name="pss", bufs=2, space="PSUM") as pss,
            tc.tile_pool(name="pso", bufs=2, space="PSUM") as pso,
        ):
            # ---- load weights / inputs ----
            def load(t_dram, shape, tag, rows=None, dt=f32r):
                t = cp.tile(shape, dt, tag=tag, name=tag)
                src = t_dram.ap()
                if dt == f32r:
                    src = src.bitcast(f32r)
                if rows is not None:
                    src = src[rows[0] : rows[1], :]
                nc.sync.dma_start(t[:], src)
                return t

            def load_bcast(t_dram, shape, tag):
                t = cp.tile(shape, f32, tag=tag, name=tag)
                nc.sync.dma_start(t[:], t_dram.ap().partition_broadcast(shape[0]))
                return t

            wq_sb = [load(wq, [128, 256], f"wq{i}", (i * 128, (i + 1) * 128)) for i in range(2)]
            wkv_sb = [load(wkv, [128, 128], f"wkv{i}", (i * 128, (i + 1) * 128)) for i in range(2)]
            wp_sb = [load(wp, [128, 256], f"wp{i}", (i * 128, (i + 1) * 128)) for i in range(2)]
            bq_bc = load_bcast(bq, [128, 512], "bq_bc")
            bkv_bc2 = load_bcast(bkv, [128, 512], "bkv_bc2")
            bp_bc = load_bcast(bp, [128, 512], "bp_bc")
            bkvc_sb = cp.tile([128, 1], f32, tag="bkvc", name="bkvc")
            nc.sync.dma_start(bkvc_sb[:], bkvc.ap())
            def load_split(t_dram, shape, tag, row0, ncol, piece, eng):
                t = cp.tile(shape, f32r, tag=tag, name=tag)
                for j0 in range(0, ncol, piece):
                    eng.dma_start(
                        t[:, j0 : j0 + piece],
                        t_dram.ap().bitcast(f32r)[
                            row0 : row0 + 128, j0 : j0 + piece])
                return t

            yk_sb = [load_split(yk, [128, 1024], f"yk{i}", i * 128, 1024, 128,
                                nc.gpsimd) for i in range(2)]
            yv_sb = [load_split(yv, [128, 1024], f"yv{i}", i * 128, 1024, 256,
                                nc.gpsimd) for i in range(2)]
            xq_sb = [load_split(xq, [128, 2048], f"xq{i}", i * 128, 2048, 256,
                                nc.scalar) for i in range(2)]

            # conv epilogue: psum has w@x; z = psum + bias (DVE), then
            # z*(1+tanh(z/2)) = 2*silu(z). Returns (z, u=z*tanh(z/2)); caller
            # emits the final add into the attention-dtype destination.
            def silu2(ps, bias_bc, tag, pcol=None):
                shape = list(ps.shape)
                z = actt.tile(shape, f32, tag="silu_z", name=f"z_{tag}")
                t = actt.tile(shape, f32, tag="silu_t", name=f"t_{tag}")
                u = actt.tile(shape, f32, tag="silu_u", name=f"u_{tag}")
                if pcol is not None:
                    nc.vector.tensor_scalar_add(z[:], ps, pcol)
                else:
                    nc.vector.tensor_add(z[:], ps, bias_bc)
                nc.scalar.activation(t[:], z[:], AF.Tanh, scale=0.5)
                nc.vector.tensor_mul(u[:], z[:], t[:])
                return z, u

            # ---- kv conv (k part): kT[pp, m0, m1], pp = hl*64+d ----
            kT = cp.tile([128, 8, 128], adt, tag="kT")
            for mt in range(2):  # m0 quads
                ps = psc.tile([128, 512], f32, tag="cnv")
                for mi in range(4):
                    m0 = 4 * mt + mi
                    for c0 in range(2):
                        nc.tensor.matmul(
                            ps[:, mi * 128 : (mi + 1) * 128],
                            lhsT=yk_sb[c0][:, m0 * 128 : (m0 + 1) * 128],
                            rhs=wkv_sb[c0][:],
                            start=(c0 == 0), stop=(c0 == 1))
                z, u = silu2(ps[:], bkv_bc2[:], f"k{mt}")
                nc.vector.tensor_add(
                    kT[:, 4 * mt : 4 * mt + 4, :].rearrange("p a b -> p (a b)"),
                    z[:], u[:])

            # ---- kv conv (v part): vext[m1, m0, hl, 0:64]=2v, [...,64]=2 ----
            vext = cp.tile([128, 8, 2, 65], adt, tag="vext")
            nc.sync.dma_start(vext[:], twod.ap().partition_broadcast(128))
            for jv in range(2):
                ps = psc.tile([128, 512], f32, tag="cnv")
                for c0 in range(2):
                    nc.tensor.matmul(
                        ps[:], lhsT=wkv_sb[c0][:],
                        rhs=yv_sb[c0][:, jv * 512 : (jv + 1) * 512],
                        start=(c0 == 0), stop=(c0 == 1))
                z, u = silu2(ps[:], None, f"v{jv}", pcol=bkvc_sb[:])
                nc.vector.tensor_add(
                    vext[:, jv * 4 : (jv + 1) * 4, :, 0:64],
                    z[:].rearrange("p (a h d) -> p a h d", a=4, h=2),
                    u[:].rearrange("p (a h d) -> p a h d", a=4, h=2))

            # ---- attention state (both heads) ----
            qT = cp.tile([128, 16, 256], adt, tag="qT")
            sums_row = [
                cp.tile([1, 4096], f32, tag=f"sumsrow{hl}", name=f"sumsrow{hl}")
                for hl in range(2)
            ]
            outun = [
                [cp.tile([128, 1024], f32, tag=f"outun{hl}_{i}",
                         name=f"outun{hl}_{i}") for i in range(2)]
                for hl in range(2)
            ]
            grid = [
                [cp.tile([128, 1024], f32, tag=f"grid{hl}_{i}",
                         name=f"grid{hl}_{i}") for i in range(2)]
                for hl in range(2)
            ]

            # q conv for one n0-pair
            def q_conv(t2):
                ps = psc.tile([128, 512], f32, tag="cnv", name=f"psq{t2}")
                for nn in range(2):
                    n0 = 2 * t2 + nn
                    for c0 in range(2):
                        nc.tensor.matmul(
                            ps[:, nn * 256 : (nn + 1) * 256],
                            lhsT=xq_sb[c0][:, n0 * 128 : (n0 + 1) * 128],
                            rhs=wq_sb[c0][:],
                            start=(c0 == 0), stop=(c0 == 1))
                z, u = silu2(ps[:], bq_bc[:], f"q{t2}")
                nc.vector.tensor_add(
                    qT[:, 2 * t2 : 2 * t2 + 2, :].rearrange("p a b -> p (a b)"),
                    z[:], u[:])

            for t2 in range(8):
                q_conv(t2)

            # t2-outer attention; scores for hl=0/1 sit on array row-groups
            # 0-63 / 64-127, emitted adjacently for PE row-group overlap.
            for t2 in range(8):  # n0 pair (n0 = 2*t2 + nn)
                att = {0: [], 1: []}
                for j in range(4):  # m0 = 2j + mi
                    scps = {}
                    for hl in range(2):
                        scps[hl] = pss.tile([128, 1024], f32, tag="scp",
                                            name=f"scp{hl}_{t2}_{j}")
                    for mi in range(2):
                        m0 = 2 * j + mi
                        for hl in range(2):
                            r0, r1 = hl * 64, (hl + 1) * 64
                            nc.tensor.matmul(
                                scps[hl][:, mi * 512 : (mi + 1) * 512],
                                lhsT=kT[r0:r1, m0, :],
                                rhs=qT[r0:r1, 2 * t2 : 2 * t2 + 2, :],
                                start=True, stop=True)
                    for hl in range(2):
                        a = attp.tile([128, 1024], adt, tag="att",
                                      name=f"att{hl}_{t2}_{j}")
                        # scoresT = 4*q.k ; want exp(q.k/8) -> scale 1/32
                        nc.scalar.activation(
                            a[:], scps[hl][:], AF.Exp, scale=0.03125)
                        att[hl].append(a)

                opss = {}
                for hl in range(2):
                    ops = pso.tile([65, 512], f32, tag="ops",
                                   name=f"ops{hl}_{t2}")
                    opss[hl] = ops
                    for m0 in range(8):
                        nc.tensor.matmul(
                            ops[:], lhsT=vext[:, m0, hl, :],
                            rhs=att[hl][m0 // 2][
                                :, (m0 % 2) * 512 : (m0 % 2 + 1) * 512],
                            start=(m0 == 0), stop=(m0 == 7))
                for hl in range(2):
                    ops = opss[hl]
                    nc.vector.tensor_copy(
                        sums_row[hl][0:1, t2 * 512 : (t2 + 1) * 512],
                        ops[64:65, :])
                    for nn in range(2):
                        n0 = 2 * t2 + nn
                        n3, q0 = n0 & 3, n0 >> 2
                        # o (x2) -> outun[c][band, q0-major cols] (contiguous)
                        dst = outun[hl][n3 // 2][
                            (n3 % 2) * 64 : (n3 % 2) * 64 + 64,
                            q0 * 256 : (q0 + 1) * 256]
                        nc.vector.tensor_copy(
                            dst.bitcast(f32r),
                            ops[0:64, nn * 256 : (nn + 1) * 256])

                if t2 % 2 == 0:
                    continue
                # ---- quarter q0 = t2//2 complete: normalize + proj ----
                q0 = t2 // 2
                for hl in range(2):
                    base = hl * 4096 + q0 * 1024
                    nc.gpsimd.dma_start(
                        bass.AP(tensor=rraw, offset=base,
                                ap=[[1024, 1], [1, 1024]]),
                        sums_row[hl][0:1, q0 * 1024 : (q0 + 1) * 1024])
                    srq = nrm.tile([16, 64], f32, tag="srq",
                                   name=f"srq{hl}_{q0}")
                    nc.gpsimd.dma_start(
                        srq[:],
                        bass.AP(tensor=rraw, offset=base,
                                ap=[[64, 16], [1, 64]]))
                    rcq = nrm.tile([16, 64], f32, tag="rcq",
                                   name=f"rcq{hl}_{q0}")
                    nc.vector.reciprocal(rcq[:], srq[:])
                    for n3g in range(4):
                        nc.gpsimd.dma_start(
                            bass.AP(tensor=rscr,
                                    offset=hl * 4096 + n3g * 1024 + q0 * 256,
                                    ap=[[64, 4], [1, 64]]),
                            rcq[n3g * 4 : (n3g + 1) * 4, :])
                    for c0 in range(2):
                        for bnd in range(2):
                            n3g = c0 * 2 + bnd
                            src = bass.AP(
                                tensor=rscr,
                                offset=hl * 4096 + n3g * 1024 + q0 * 256,
                                ap=[[0, 64], [1, 256]])
                            nc.gpsimd.dma_start(
                                grid[hl][c0][bnd * 64 : (bnd + 1) * 64,
                                             q0 * 256 : (q0 + 1) * 256],
                                src)
                        sl = slice(q0 * 256, (q0 + 1) * 256)
                        nc.vector.tensor_mul(
                            outun[hl][c0][:, sl].bitcast(f32r),
                            outun[hl][c0][:, sl], grid[hl][c0][:, sl])
                    # proj fc in {2q0, 2q0+1}; rows hl*1024+half*512+q0+4r
                    ps2 = psc.tile([128, 512], f32, tag="cnv",
                                   name=f"psproj{hl}_{q0}")
                    for half in range(2):
                        fc = 2 * q0 + half
                        for c0 in range(2):
                            nc.tensor.matmul(
                                ps2[:, half * 256 : (half + 1) * 256],
                                lhsT=outun[hl][c0][:].bitcast(f32r)[
                                    :, fc * 128 : (fc + 1) * 128],
                                rhs=wp_sb[c0][:],
                                start=(c0 == 0), stop=(c0 == 1))
                    osb = outp.tile([128, 512], f32, tag="osb",
                                    name=f"osb{hl}_{q0}")
                    nc.vector.tensor_add(osb[:], ps2[:], bp_bc[:])
                    dstap = bass.AP(
                        tensor=out,
                        offset=(hl * 1024 + q0) * 256,
                        ap=[[4 * 256, 128], [512 * 256, 2], [1, 256]])
                    nc.sync.dma_start(
                        dstap,
                        osb[:].rearrange("p (h c) -> p h c", h=2))

    nc.compile()
    return nc


def _prep_inputs(x, y, q_w, q_gamma, q_beta, q_mean, q_var,
                 kv_w, kv_gamma, kv_beta, kv_mean, kv_var, proj_w, proj_b):
    f = np.float32
    x = np.ascontiguousarray(np.asarray(x, f))
    y = np.ascontiguousarray(np.asarray(y, f))

    gq = np.asarray(q_gamma, f) / np.sqrt(np.asarray(q_var, f) + BN_EPS)
    bq_full = np.asarray(q_beta, f) - np.asarray(q_mean, f) * gq
    wq_host = np.ascontiguousarray((np.asarray(q_w, f) * gq[:, None]).T)

    gkv = np.asarray(kv_gamma, f) / np.sqrt(np.asarray(kv_var, f) + BN_EPS)
    bkv_full = np.asarray(kv_beta, f) - np.asarray(kv_mean, f) * gkv
    wkv_host = np.ascontiguousarray((np.asarray(kv_w, f) * gkv[:, None]).T)

    wp_host = np.ascontiguousarray(np.asarray(proj_w, f).T)
    bp_host = np.asarray(proj_b, f)

    bq2 = np.tile(bq_full[None, :], (1, 2)).astype(f)
    bkv2 = np.tile(bkv_full[None, :], (1, 4)).astype(f)
    bp2 = np.tile(bp_host[None, :], (1, 2)).astype(f)

    in_maps = []
    for core in range(8):
        b, hp = core // 2, core % 2
        X4 = x[b].reshape(C, N_TOK)
        Y4 = y[b].reshape(C, N_TOK)
        xqa = np.ascontiguousarray(
            X4.reshape(C, 16, 256)[:, :, hp * 128 : (hp + 1) * 128]).reshape(C, 2048)
        Y8 = Y4.reshape(C, 8, 512)
        yka = np.ascontiguousarray(
            Y8[:, :, hp * 128 : (hp + 1) * 128]).reshape(C, 1024)
        yva = np.ascontiguousarray(
            Y8[:, :, 256 + hp * 128 : 256 + (hp + 1) * 128]).reshape(C, 1024)
        in_maps.append({
            "xq": xqa, "yk": yka, "yv": yva,
            "wq": wq_host, "bq": bq2,
            "wkv": wkv_host, "bkv": bkv2,
            "bkvc": bkv_full[:, None].astype(f),
            "wp": wp_host, "bp": bp2,
            "twod": np.full((1, 1040), 2.0,
                            ml_dtypes.bfloat16 if ATTN_BF16 else f),
        })
    return in_maps


def _get_nc():
    if "nc" not in _CACHE:
        _CACHE["nc"] = _build()
    return _CACHE["nc"]


def kernel(x, y, H=64, W=64, q_w=None, q_gamma=None, q_beta=None, q_mean=None,
           q_var=None, kv_w=None, kv_gamma=None, kv_beta=None, kv_mean=None,
           kv_var=None, proj_w=None, proj_b=None, _trace=False):
    from concourse.bass_utils import run_bass_kernel_spmd

    nc = _get_nc()
    in_maps = _prep_inputs(x, y, q_w, q_gamma, q_beta, q_mean, q_var,
                           kv_w, kv_gamma, kv_beta, kv_mean, kv_var,
                           proj_w, proj_b)
    kw = {}
    if _trace:
        kw = {"trace": True, "trace_cores": list(range(8))}
    res = run_bass_kernel_spmd(nc, in_maps, list(range(8)), **kw)
    outa = np.empty((B, N_TOK, C), np.float32)
    for core in range(8):
        b, hp = core // 2, core % 2
        outa[b, hp * 2048 : (hp + 1) * 2048, :] = res.results[core]["out"]
    if _trace:
        return outa, res
    return outa


# revision 35
# speedup vs baseline: 1.0353x; 1.0353x over previous
"""Trainium2 Bass kernel for nn_Attention_57423712748130.

Computation (per batch b):
  X4 = x[b] viewed (C=256, N=4096)   [raw reshape]
  Q4 = silu(BN(q_w @ X4))            (256, 4096)
  KV4 = silu(BN(kv_w @ Y4))          (128, 4096)
  q[n,h,d]  = Q4[n1, n0*256+h*64+d]      n = n1*16+n0
  k[m,h,d]  = KV4[m1, m0*512 + h*64+d]   m = m1*8+m0
  v[m,h,d]  = KV4[m1, m0*512+256+h*64+d]
  att = softmax(q k^T / 8); o = att v
  out rows [h*1024,(h+1)*1024) = O_h @ proj_w.T + proj_b
    where O_h[n2, n3*64+d] = o[4*n2+n3, d]

Sharding: 8 cores = (batch b in 0..3) x (head-pair hp in 0..1); each core
computes heads {2hp, 2hp+1} of batch b = rows [hp*2048,(hp+1)*2048) of out[b].

On-core strategy:
 - conv outputs are computed directly in transposed layout (x/y tiles as the
   matmul stationary), so q^T/k^T slices ([d on partitions]) need no transposes
 - conv bias is applied on VectorE (a K=1 bias matmul costs ~500ns of PE each);
   silu is computed as z*(1+tanh(z/2)) = 2*silu(z) — tanh shares the ACT table
   set with exp (plain Silu thrashes ACT_TABLE_LOADs against Exp); the 2x is
   folded: exp scale 1/32 and a 2.0 fill for the V/ones column cancel it
 - scoresT[m,n] = k^T.T @ q^T in PSUM, one [128,1024] exp per 2-bank psum
 - att@v contracts over m with an extra ones-column on V producing the softmax
   denominators as row 64; reciprocal runs as one [16,64] DVE op per quarter
   (a per-n0 [1,256] reciprocal is 1.75us of iterative divide each)
 - scores/att@v operands are bf16 (PSUM accumulation fp32); convs/proj f32r
   (f32r matmuls must self-load weights so their LDWEIGHTS serializes; bf16
   LDWEIGHTS overlaps ~99% with matmuls). measured end-to-end absmax-rel
   ~3.1e-3, resid_var ~1.5e-6; measured ~151-170us across 8 cores
 - normalization + projection run per q0-quarter, pipelined behind the
   attention of later quarters (grids broadcast via a DRAM bounce); O_h
   columns are kept q0-major so every PSUM evacuation is contiguous; the
   final output DMA un-permutes rows
"""

import ml_dtypes
import numpy as np

B = 4
N_TOK = 4096
C = 256
BN_EPS = 1e-5
ATTN_BF16 = True  # scores/att@v operands in bf16 (PSUM accum stays fp32)

_CACHE = {}


def _build():
    import concourse.bacc as bacc
    import concourse.bass as bass
    import concourse.tile as tile
    from concourse import mybir

    f32 = mybir.dt.float32
    f32r = mybir.dt.float32r
    bf16 = mybir.dt.bfloat16
    adt = bf16 if ATTN_BF16 else f32r
    AF = mybir.ActivationFunctionType

    nc = bacc.Bacc("TRN2", target_bir_lowering=False, debug=False, num_devices=8)

    xq = nc.dram_tensor("xq", [256, 2048], f32, kind="ExternalInput")
    yk = nc.dram_tensor("yk", [256, 1024], f32, kind="ExternalInput")
    yv = nc.dram_tensor("yv", [256, 1024], f32, kind="ExternalInput")
    wq = nc.dram_tensor("wq", [256, 256], f32, kind="ExternalInput")
    bq = nc.dram_tensor("bq", [1, 512], f32, kind="ExternalInput")
    wkv = nc.dram_tensor("wkv", [256, 128], f32, kind="ExternalInput")
    bkv = nc.dram_tensor("bkv", [1, 512], f32, kind="ExternalInput")
    bkvc = nc.dram_tensor("bkvc", [128, 1], f32, kind="ExternalInput")
    wp = nc.dram_tensor("wp", [256, 256], f32, kind="ExternalInput")
    bp = nc.dram_tensor("bp", [1, 512], f32, kind="ExternalInput")
    twod = nc.dram_tensor("twod", [1, 1040], adt, kind="ExternalInput")
    out = nc.dram_tensor("out", [2048, 256], f32, kind="ExternalOutput")
    rscr = nc.dram_tensor("rscr", [2, 4096], f32)
    rraw = nc.dram_tensor("rraw", [2, 4096], f32)

    with tile.TileContext(nc) as tc:
        with (
            tc.tile_pool(name="const", bufs=1) as cp,
            tc.tile_pool(name="actt", bufs=3) as actt,
            tc.tile_pool(name="attp", bufs=16) as attp,
            tc.tile_pool(name="outp", bufs=4) as outp,
            tc.tile_pool(name="nrm", bufs=4) as nrm,
            tc.tile_pool(name="psc", bufs=1, space="PSUM") as psc,
            tc.tile_pool(name="pss", bufs=2, space="PSUM") as pss,
            tc.tile_pool(name="pso", bufs=3, space="PSUM") as pso,
        ):
            # ---- load weights / inputs ----
            def load(t_dram, shape, tag, rows=None, dt=f32r):
                t = cp.tile(shape, dt, tag=tag, name=tag)
                src = t_dram.ap()
                if dt == f32r:
                    src = src.bitcast(f32r)
                if rows is not None:
                    src = src[rows[0] : rows[1], :]
                nc.sync.dma_start(t[:], src)
                return t

            def load_bcast(t_dram, shape, tag):
                t = cp.tile(shape, f32, tag=tag, name=tag)
                nc.sync.dma_start(t[:], t_dram.ap().partition_broadcast(shape[0]))
                return t

            wq_sb = [load(wq, [128, 256], f"wq{i}", (i * 128, (i + 1) * 128)) for i in range(2)]
            wkv_sb = [load(wkv, [128, 128], f"wkv{i}", (i * 128, (i + 1) * 128)) for i in range(2)]
            wp_sb = [load(wp, [128, 256], f"wp{i}", (i * 128, (i + 1) * 128)) for i in range(2)]
            bq_bc = load_bcast(bq, [128, 512], "bq_bc")
            bkv_bc2 = load_bcast(bkv, [128, 512], "bkv_bc2")
            bp_bc = load_bcast(bp, [128, 512], "bp_bc")
            bkvc_sb = cp.tile([128, 1], f32, tag="bkvc", name="bkvc")
            nc.sync.dma_start(bkvc_sb[:], bkvc.ap())
            def load_split(t_dram, shape, tag, row0, ncol, piece, eng):
                t = cp.tile(shape, f32r, tag=tag, name=tag)
                for j0 in range(0, ncol, piece):
                    eng.dma_start(
                        t[:, j0 : j0 + piece],
                        t_dram.ap().bitcast(f32r)[
                            row0 : row0 + 128, j0 : j0 + piece])
                return t

            yk_sb = [load_split(yk, [128, 1024], f"yk{i}", i * 128, 1024, 128,
                                nc.gpsimd) for i in range(2)]
            yv_sb = [load_split(yv, [128, 1024], f"yv{i}", i * 128, 1024, 256,
                                nc.gpsimd) for i in range(2)]
            xq_sb = [load_split(xq, [128, 2048], f"xq{i}", i * 128, 2048, 256,
                                nc.scalar) for i in range(2)]

            # conv epilogue: psum has w@x; z = psum + bias (DVE), then
            # z*(1+tanh(z/2)) = 2*silu(z). Returns (z, u=z*tanh(z/2)); caller
            # emits the final add into the attention-dtype destination.
            def silu2(ps, bias_bc, tag, pcol=None):
                shape = list(ps.shape)
                z = actt.tile(shape, f32, tag="silu_z", name=f"z_{tag}")
                t = actt.tile(shape, f32, tag="silu_t", name=f"t_{tag}")
                u = actt.tile(shape, f32, tag="silu_u", name=f"u_{tag}")
                if pcol is not None:
                    nc.vector.tensor_scalar_add(z[:], ps, pcol)
                else:
                    nc.vector.tensor_add(z[:], ps, bias_bc)
                nc.scalar.activation(t[:], z[:], AF.Tanh, scale=0.5)
                nc.vector.tensor_mul(u[:], z[:], t[:])
                return z, u

            # ---- kv conv (k part): kT[pp, m0, m1], pp = hl*64+d ----
            kT = cp.tile([128, 8, 128], adt, tag="kT")
            for mt in range(2):  # m0 quads
                ps = psc.tile([128, 512], f32, tag="cnv")
                for mi in range(4):
                    m0 = 4 * mt + mi
                    for c0 in range(2):
                        nc.tensor.matmul(
                            ps[:, mi * 128 : (mi + 1) * 128],
                            lhsT=yk_sb[c0][:, m0 * 128 : (m0 + 1) * 128],
                            rhs=wkv_sb[c0][:],
                            start=(c0 == 0), stop=(c0 == 1))
                z, u = silu2(ps[:], bkv_bc2[:], f"k{mt}")
                nc.vector.tensor_add(
                    kT[:, 4 * mt : 4 * mt + 4, :].rearrange("p a b -> p (a b)"),
                    z[:], u[:])

            # ---- kv conv (v part): vext[m1, m0, hl, 0:64]=2v, [...,64]=2 ----
            vext = cp.tile([128, 8, 2, 65], adt, tag="vext")
            nc.sync.dma_start(vext[:], twod.ap().partition_broadcast(128))
            for jv in range(2):
                ps = psc.tile([128, 512], f32, tag="cnv")
                for c0 in range(2):
                    nc.tensor.matmul(
                        ps[:], lhsT=wkv_sb[c0][:],
                        rhs=yv_sb[c0][:, jv * 512 : (jv + 1) * 512],
                        start=(c0 == 0), stop=(c0 == 1))
                z, u = silu2(ps[:], None, f"v{jv}", pcol=bkvc_sb[:])
                nc.vector.tensor_add(
                    vext[:, jv * 4 : (jv + 1) * 4, :, 0:64],
                    z[:].rearrange("p (a h d) -> p a h d", a=4, h=2),
                    u[:].rearrange("p (a h d) -> p a h d", a=4, h=2))

            # ---- attention state (both heads) ----
            qT = cp.tile([128, 16, 256], adt, tag="qT")
            sums_row = [
                cp.tile([1, 4096], f32, tag=f"sumsrow{hl}", name=f"sumsrow{hl}")
                for hl in range(2)
            ]
            outun = [
                [cp.tile([128, 1024], f32, tag=f"outun{hl}_{i}",
                         name=f"outun{hl}_{i}") for i in range(2)]
                for hl in range(2)
            ]
            grid = [
                [cp.tile([128, 1024], f32, tag=f"grid{hl}_{i}",
                         name=f"grid{hl}_{i}") for i in range(2)]
                for hl in range(2)
            ]

            # q conv for one n0-pair
            def q_conv(t2):
                ps = psc.tile([128, 512], f32, tag="cnv", name=f"psq{t2}")
                for nn in range(2):
                    n0 = 2 * t2 + nn
                    for c0 in range(2):
                        nc.tensor.matmul(
                            ps[:, nn * 256 : (nn + 1) * 256],
                            lhsT=xq_sb[c0][:, n0 * 128 : (n0 + 1) * 128],
                            rhs=wq_sb[c0][:],
                            start=(c0 == 0), stop=(c0 == 1))
                z, u = silu2(ps[:], bq_bc[:], f"q{t2}")
                nc.vector.tensor_add(
                    qT[:, 2 * t2 : 2 * t2 + 2, :].rearrange("p a b -> p (a b)"),
                    z[:], u[:])

            for t2 in range(8):
                q_conv(t2)

            # t2-outer attention; scores for hl=0/1 sit on array row-groups
            # 0-63 / 64-127, emitted adjacently for PE row-group overlap.
            for t2 in range(8):  # n0 pair (n0 = 2*t2 + nn)
                att = {0: [], 1: []}
                for j in range(4):  # m0 = 2j + mi
                    scps = {}
                    for hl in range(2):
                        scps[hl] = pss.tile([128, 1024], f32, tag="scp",
                                            name=f"scp{hl}_{t2}_{j}")
                    for mi in range(2):
                        m0 = 2 * j + mi
                        for hl in range(2):
                            r0, r1 = hl * 64, (hl + 1) * 64
                            nc.tensor.matmul(
                                scps[hl][:, mi * 512 : (mi + 1) * 512],
                                lhsT=kT[r0:r1, m0, :],
                                rhs=qT[r0:r1, 2 * t2 : 2 * t2 + 2, :],
                                start=True, stop=True)
                    for hl in range(2):
                        a = attp.tile([128, 1024], adt, tag="att",
                                      name=f"att{hl}_{t2}_{j}")
                        # scoresT = 4*q.k ; want exp(q.k/8) -> scale 1/32
                        nc.scalar.activation(
                            a[:], scps[hl][:], AF.Exp, scale=0.03125)
                        att[hl].append(a)

                opss = {}
                for hl in range(2):
                    ops = pso.tile([65, 512], f32, tag="ops",
                                   name=f"ops{hl}_{t2}")
                    opss[hl] = ops
                    for m0 in range(8):
                        nc.tensor.matmul(
                            ops[:], lhsT=vext[:, m0, hl, :],
                            rhs=att[hl][m0 // 2][
                                :, (m0 % 2) * 512 : (m0 % 2 + 1) * 512],
                            start=(m0 == 0), stop=(m0 == 7))
                for hl in range(2):
                    ops = opss[hl]
                    nc.vector.tensor_copy(
                        sums_row[hl][0:1, t2 * 512 : (t2 + 1) * 512],
                        ops[64:65, :])
                    for nn in range(2):
                        n0 = 2 * t2 + nn
                        n3, q0 = n0 & 3, n0 >> 2
                        # o (x2) -> outun[c][band, q0-major cols] (contiguous)
                        dst = outun[hl][n3 // 2][
                            (n3 % 2) * 64 : (n3 % 2) * 64 + 64,
                            q0 * 256 : (q0 + 1) * 256]
                        nc.vector.tensor_copy(
                            dst.bitcast(f32r),
                            ops[0:64, nn * 256 : (nn + 1) * 256])

                if t2 % 2 == 0:
                    continue
                # ---- quarter q0 = t2//2 complete: normalize + proj ----
                q0 = t2 // 2
                for hl in range(2):
                    base = hl * 4096 + q0 * 1024
                    nc.gpsimd.dma_start(
                        bass.AP(tensor=rraw, offset=base,
                                ap=[[1024, 1], [1, 1024]]),
                        sums_row[hl][0:1, q0 * 1024 : (q0 + 1) * 1024])
                    srq = nrm.tile([16, 64], f32, tag="srq",
                                   name=f"srq{hl}_{q0}")
                    nc.gpsimd.dma_start(
                        srq[:],
                        bass.AP(tensor=rraw, offset=base,
                                ap=[[64, 16], [1, 64]]))
                    rcq = nrm.tile([16, 64], f32, tag="rcq",
                                   name=f"rcq{hl}_{q0}")
                    nc.vector.reciprocal(rcq[:], srq[:])
                    for n3g in range(4):
                        nc.gpsimd.dma_start(
                            bass.AP(tensor=rscr,
                                    offset=hl * 4096 + n3g * 1024 + q0 * 256,
                                    ap=[[64, 4], [1, 64]]),
                            rcq[n3g * 4 : (n3g + 1) * 4, :])
                    for c0 in range(2):
                        for bnd in range(2):
                            n3g = c0 * 2 + bnd
                            src = bass.AP(
                                tensor=rscr,
                                offset=hl * 4096 + n3g * 1024 + q0 * 256,
                                ap=[[0, 64], [1, 256]])
                            nc.gpsimd.dma_start(
                                grid[hl][c0][bnd * 64 : (bnd + 1) * 64,
                                             q0 * 256 : (q0 + 1) * 256],
                                src)
                        sl = slice(q0 * 256, (q0 + 1) * 256)
                        nc.vector.tensor_mul(
                            outun[hl][c0][:, sl].bitcast(f32r),
                            outun[hl][c0][:, sl], grid[hl][c0][:, sl])
                    # proj fc in {2q0, 2q0+1}; rows hl*1024+half*512+q0+4r
                    ps2 = psc.tile([128, 512], f32, tag="cnv",
                                   name=f"psproj{hl}_{q0}")
                    for half in range(2):
                        fc = 2 * q0 + half
                        for c0 in range(2):
                            nc.tensor.matmul(
                                ps2[:, half * 256 : (half + 1) * 256],
                                lhsT=outun[hl][c0][:].bitcast(f32r)[
                                    :, fc * 128 : (fc + 1) * 128],
                                rhs=wp_sb[c0][:],
                                start=(c0 == 0), stop=(c0 == 1))
                    osb = outp.tile([128, 512], f32, tag="osb",
                                    name=f"osb{hl}_{q0}")
                    nc.vector.tensor_add(osb[:], ps2[:], bp_bc[:])
                    dstap = bass.AP(
                        tensor=out,
                        offset=(hl * 1024 + q0) * 256,
                        ap=[[4 * 256, 128], [512 * 256, 2], [1, 256]])
                    nc.sync.dma_start(
                        dstap,
                        osb[:].rearrange("p (h c) -> p h c", h=2))

    nc.compile()
    return nc


def _prep_inputs(x, y, q_w, q_gamma, q_beta, q_mean, q_var,
                 kv_w, kv_gamma, kv_beta, kv_mean, kv_var, proj_w, proj_b):
    f = np.float32
    x = np.ascontiguousarray(np.asarray(x, f))
    y = np.ascontiguousarray(np.asarray(y, f))

    gq = np.asarray(q_gamma, f) / np.sqrt(np.asarray(q_var, f) + BN_EPS)
    bq_full = np.asarray(q_beta, f) - np.asarray(q_mean, f) * gq
    wq_host = np.ascontiguousarray((np.asarray(q_w, f) * gq[:, None]).T)

    gkv = np.asarray(kv_gamma, f) / np.sqrt(np.asarray(kv_var, f) + BN_EPS)
    bkv_full = np.asarray(kv_beta, f) - np.asarray(kv_mean, f) * gkv
    wkv_host = np.ascontiguousarray((np.asarray(kv_w, f) * gkv[:, None]).T)

    wp_host = np.ascontiguousarray(np.asarray(proj_w, f).T)
    bp_host = np.asarray(proj_b, f)

    bq2 = np.tile(bq_full[None, :], (1, 2)).astype(f)
    bkv2 = np.tile(bkv_full[None, :], (1, 4)).astype(f)
    bp2 = np.tile(bp_host[None, :], (1, 2)).astype(f)

    in_maps = []
    for core in range(8):
        b, hp = core // 2, core % 2
        X4 = x[b].reshape(C, N_TOK)
        Y4 = y[b].reshape(C, N_TOK)
        xqa = np.ascontiguousarray(
            X4.reshape(C, 16, 256)[:, :, hp * 128 : (hp + 1) * 128]).reshape(C, 2048)
        Y8 = Y4.reshape(C, 8, 512)
        yka = np.ascontiguousarray(
            Y8[:, :, hp * 128 : (hp + 1) * 128]).reshape(C, 1024)
        yva = np.ascontiguousarray(
            Y8[:, :, 256 + hp * 128 : 256 + (hp + 1) * 128]).reshape(C, 1024)
        in_maps.append({
            "xq": xqa, "yk": yka, "yv": yva,
            "wq": wq_host, "bq": bq2,
            "wkv": wkv_host, "bkv": bkv2,
            "bkvc": bkv_full[:, None].astype(f),
            "wp": wp_host, "bp": bp2,
            "twod": np.full((1, 1040), 2.0,
                            ml_dtypes.bfloat16 if ATTN_BF16 else f),
        })
    return in_maps


def _get_nc():
    if "nc" not in _CACHE:
        _CACHE["nc"] = _build()
    return _CACHE["nc"]


def kernel(x, y, H=64, W=64, q_w=None, q_gamma=None, q_beta=None, q_mean=None,
           q_var=None, kv_w=None, kv_gamma=None, kv_beta=None, kv_mean=None,
           kv_var=None, proj_w=None, proj_b=None, _trace=False):
    from concourse.bass_utils import run_bass_kernel_spmd

    nc = _get_nc()
    in_maps = _prep_inputs(x, y, q_w, q_gamma, q_beta, q_mean, q_var,
                           kv_w, kv_gamma, kv_beta, kv_mean, kv_var,
                           proj_w, proj_b)
    kw = {}
    if _trace:
        kw = {"trace": True, "trace_cores": list(range(8))}
    res = run_bass_kernel_spmd(nc, in_maps, list(range(8)), **kw)
    outa = np.empty((B, N_TOK, C), np.float32)
    for core in range(8):
        b, hp = core // 2, core % 2
        outa[b, hp * 2048 : (hp + 1) * 2048, :] = res.results[core]["out"]
    if _trace:
        return outa, res
    return outa
